# revision 1
# baseline (speedup 1.0000x reference)
"""Trainium2 Bass kernel for nn_MultiHeadAttention_22883585753377.

Reference semantics (torch legacy): softmax over the HEADS axis (dim=1) of
the [B,H,S,S] score tensor, scale = sqrt(KEY_DIM)=32.

Sharding: 8 cores = (batch b, query-quarter). Each core handles b = c//4 and
512 query rows, all 16 heads (the heads-softmax couples heads, so they stay
local). Everything else is computed on-device per core:
  Q/K/V projections (bf16 matmuls), scores^T per head in [k-part, q-free]
  layout, exp on ScalarE (PSUM->SBUF, scale folded), cross-head denominator
  via DVE tree adds, reciprocal_approx_fast, weights multiply, AV matmul with
  col-packed head pairs, then the output projection.

Host-side work is layout only: transpose/cast/shard inputs, concat outputs.
"""

import numpy as np

B = 2
S = 1024 * 2
D = 1024
H = 16
DH = 64
SQ = 512  # query rows per core
QH = 256  # q processed per half
KC = 128  # k-chunk (partition dim of scores^T tiles)
NKC = S // KC  # 16
SCALE = 1.0 / 32.0  # 1/sqrt(KEY_DIM)

_CACHE = {}


def _legalize_waits(nc):
    """This container's walrus encodes at most ONE semaphore wait per
    instruction; Tile emits up to ~10. Split the excess onto same-engine nops
    inserted immediately before the instruction. An engine's sequencer blocks
    at the same stream position either way, so ordering semantics are
    preserved; every wait references a producer earlier in Tile's schedule, so
    prefix-executability (deadlock freedom) is retained."""
    import bass_rust

    ctr = [0]
    for bb in nc.main_func.blocks:
        insts = list(bb.instructions)
        out = []
        changed = False
        for ins in insts:
            si = ins.sync_info
            waits = list(si.on_wait) if si is not None and si.on_wait else []
            if len(waits) > 1:
                changed = True
                upd = list(si.on_update) if si.on_update else []
                for w in waits[:-1]:
                    ctr[0] += 1
                    nop = bass_rust.InstNoOp(
                        name=f"I-wsplit-{ctr[0]}", ins=[], outs=[]
                    )
                    nop.engine = ins.engine
                    nop.bass_nofuse = True
                    nop.sync_info = bass_rust.SyncInfo(on_wait=[w], on_update=[])
                    out.append(nop)
                ins.sync_info = bass_rust.SyncInfo(
                    on_wait=[waits[-1]], on_update=upd
                )
            out.append(ins)
        if changed:
            bb.instructions = out


def _build(legalize=True, nkc=NKC, nqh=2, phase_c=True, do_k=True, do_v=True, do_attn=True, attn_stop='full'):
    import concourse.bass as bass
    import concourse.mybir as mybir
    import concourse.tile as tile

    bf16 = mybir.dt.bfloat16
    f32 = mybir.dt.float32
    AF = mybir.ActivationFunctionType

    nc = bass.Bass()

    # --- I/O ---------------------------------------------------------------
    qT_d = nc.dram_tensor("qT", [D, SQ], bf16, kind="ExternalInput")
    kT_d = nc.dram_tensor("kT", [D, S], bf16, kind="ExternalInput")
    vT_d = nc.dram_tensor("vT", [D, S], bf16, kind="ExternalInput")
    wq_d = nc.dram_tensor("wq", [D, D], bf16, kind="ExternalInput")
    wk_d = nc.dram_tensor("wk", [D, D], bf16, kind="ExternalInput")
    wv_d = nc.dram_tensor("wv", [D, D], bf16, kind="ExternalInput")
    wo_d = nc.dram_tensor("wo", [D, D], bf16, kind="ExternalInput")
    bqr_d = nc.dram_tensor("bqr", [128, 8], mybir.dt.float32, kind="ExternalInput")
    bkr_d = nc.dram_tensor("bkr", [128, 8], mybir.dt.float32, kind="ExternalInput")
    bq_d = nc.dram_tensor("bq", [1, D], bf16, kind="ExternalInput")
    bk_d = nc.dram_tensor("bk", [1, D], bf16, kind="ExternalInput")
    bv_d = nc.dram_tensor("bv", [1, D], bf16, kind="ExternalInput")
    bo_d = nc.dram_tensor("bo", [1, D], bf16, kind="ExternalInput")
    out_d = nc.dram_tensor("out", [SQ, D], f32, kind="ExternalOutput")

    with tile.TileContext(nc) as tc:
        # Long-lived SBUF tensors.
        with (
            tc.tile_pool(name="persist", bufs=1) as persist,
            tc.tile_pool(name="consts", bufs=1) as consts,
        ):
            # K^T [D,S] as 8 x [128, S]; partition chunk p holds heads 2p,2p+1
            KT = [persist.tile([128, S], bf16, tag=f"KT{p}", name=f"KT{p}") for p in range(8)]
            # V natural [S,D] as 16 x [128, D]
            V = [persist.tile([128, D], bf16, tag=f"V{s}", name=f"V{s}") for s in range(16)]
            # Q^T per head, zero-padded to the full pair-chunk: tile h holds
            # head h's 64 features at rows (h%2)*64 and ZEROS on the other 64
            # rows. Scores matmuls then use the full 128-row contraction with
            # the paired K^T tile -- the zero rows annihilate the other head.
            # (64-row contractions alternating between base partitions 0 and
            # 64 with a 128-wide stationary operand crash this runtime.)
            QT = [persist.tile([128, SQ], bf16, tag=f"QT{p}", name=f"QT{p}") for p in range(16)]
            # O^T [D,SQ] as 8 x [128, SQ] (pair j = heads 2j,2j+1)
            OT = [persist.tile([128, SQ], bf16, tag=f"OT{p}", name=f"OT{p}") for p in range(8)]

            ones = consts.tile([1, S], bf16)
            nc.vector.memset(ones[:], 1.0)
            bqr_s = consts.tile([128, 8], f32, tag="bqr")
            bkr_s = consts.tile([128, 8], f32, tag="bkr")
            nc.sync.dma_start(bqr_s[:], bqr_d[:])
            nc.sync.dma_start(bkr_s[:], bkr_d[:])
            bq_s = consts.tile([1, D], bf16, tag="bq")
            bk_s = consts.tile([1, D], bf16, tag="bk")
            bv_s = consts.tile([1, D], bf16, tag="bv")
            bo_s = consts.tile([1, D], bf16, tag="bo")
            nc.sync.dma_start(bq_s[:], bq_d[:])
            nc.sync.dma_start(bk_s[:], bk_d[:])
            nc.sync.dma_start(bv_s[:], bv_d[:])
            nc.sync.dma_start(bo_s[:], bo_d[:])

            # ---------------- Phase A: projections -------------------------
            with (
                tc.tile_pool(name="raw", bufs=1) as raw,
                tc.tile_pool(name="wrow", bufs=1) as wrow,
                tc.tile_pool(name="pA", bufs=4, space="PSUM") as pA,
                tc.tile_pool(name="vsl", bufs=3) as vsl,
            ):
                wqr = [wrow.tile([128, D], bf16, tag=f"wqr{d}", name=f"wqr{d}") for d in range(8)]
                wkr = [wrow.tile([128, D], bf16, tag=f"wkr{d}", name=f"wkr{d}") for d in range(8)]
                wvr = [wrow.tile([128, D], bf16, tag=f"wvr{d}", name=f"wvr{d}") for d in range(8)]
                for d in range(8):
                    nc.sync.dma_start(wqr[d][:], wq_d[d * 128 : (d + 1) * 128, :])
                    nc.sync.dma_start(wkr[d][:], wk_d[d * 128 : (d + 1) * 128, :])
                    nc.sync.dma_start(wvr[d][:], wv_d[d * 128 : (d + 1) * 128, :])

                # Q projection: QT[f,q] = sum_d wq[d,f]*qT[d,q] (+ bq)
                for h in range(16):
                    r = (h % 2) * 64
                    nc.vector.memset(QT[h][64 - r : 128 - r, :], 0.0)
                qraw = [raw.tile([128, SQ], bf16, tag=f"qraw{d}", name=f"qraw{d}") for d in range(8)]
                for d in range(8):
                    nc.sync.dma_start(qraw[d][:], qT_d[d * 128 : (d + 1) * 128, :])
                for f in range(8):
                    ps = pA.tile([128, SQ], f32, tag="pj", bufs=4)
                    for d in range(8):
                        nc.tensor.matmul(
                            ps[:],
                            wqr[d][:, f * 128 : (f + 1) * 128],
                            qraw[d][:],
                            start=(d == 0),
                            stop=(d == 7),
                        )
                    nc.scalar.activation(
                        QT[2 * f][0:64, :], ps[0:64, :], AF.Identity,
                        bias=bqr_s[0:64, f : f + 1],
                    )
                    nc.scalar.activation(
                        QT[2 * f + 1][64:128, :], ps[64:128, :], AF.Identity,
                        bias=bqr_s[64:128, f : f + 1],
                    )

                # K projection (full, per core): KT[f,k] = sum_d wk[d,f]*kT[d,k]
                kraw = [raw.tile([128, S], bf16, tag=f"kraw{d}", name=f"kraw{d}") for d in range(8)]
                for d in range(8):
                    nc.sync.dma_start(kraw[d][:], kT_d[d * 128 : (d + 1) * 128, :])
                for f in range(8):
                    pk = [
                        pA.tile([128, 512], f32, tag="pj", bufs=4, name=f"pk{k4}")
                        for k4 in range(4)
                    ]
                    for d in range(8):
                        for k4 in range(4):
                            nc.tensor.matmul(
                                pk[k4][:],
                                wkr[d][:, f * 128 : (f + 1) * 128],
                                kraw[d][:, k4 * 512 : (k4 + 1) * 512],
                                start=(d == 0),
                                stop=(d == 7),
                            )
                    for k4 in range(4):
                        nc.scalar.activation(
                            KT[f][:, k4 * 512 : (k4 + 1) * 512], pk[k4][:],
                            AF.Identity, bias=bkr_s[:, f : f + 1],
                        )

                # V projection (full): V[s,f] = sum_d values[s,d]*wv[d,f] (+bv)
                for sg in range(8):
                    pv = [pA.tile([128, 512], f32, tag=f"psv{i}", name=f"psv{i}", bufs=1) for i in range(4)]
                    for d in range(8):
                        vs = vsl.tile([128, 256], bf16, tag="vsl")
                        nc.sync.dma_start(
                            vs[:],
                            vT_d[d * 128 : (d + 1) * 128, sg * 256 : (sg + 1) * 256],
                        )
                        for s2 in range(2):
                            for f2 in range(2):
                                nc.tensor.matmul(
                                    pv[s2 * 2 + f2][:],
                                    vs[:, s2 * 128 : (s2 + 1) * 128],
                                    wvr[d][:, f2 * 512 : (f2 + 1) * 512],
                                    start=(d == 0),
                                    stop=False,
                                )
                    for s2 in range(2):
                        for f2 in range(2):
                            nc.tensor.matmul(
                                pv[s2 * 2 + f2][:],
                                ones[0:1, 0:128],
                                bv_s[0:1, f2 * 512 : (f2 + 1) * 512],
                                start=False,
                                stop=True,
                            )
                            nc.scalar.copy(
                                V[sg * 2 + s2][:, f2 * 512 : (f2 + 1) * 512],
                                pv[s2 * 2 + f2][:],
                            )

            # ---------------- Phase B: attention ---------------------------
            with (
                tc.tile_pool(name="psc", bufs=2, space="PSUM") as psc,
                tc.tile_pool(name="pav", bufs=1, space="PSUM") as pav,
                tc.tile_pool(name="exp", bufs=3) as expp,
                tc.tile_pool(name="wts", bufs=3) as wtsp,
                tc.tile_pool(name="mid", bufs=4) as mid,
            ):
                for qh in (range(nqh) if do_attn else range(0)):
                    qsl = slice(qh * QH, (qh + 1) * QH)
                    oacc = [
                        pav.tile([128, 2 * QH], f32, tag=f"oacc{i}", name=f"oacc{i}", bufs=1)
                        for i in range(4)
                    ]
                    for kc in range(nkc):
                        kcs = slice(kc * 128, (kc + 1) * 128)
                        # scores^T for all 16 heads -> exp tile [128, 16*QH]
                        e = expp.tile([128, H * QH], bf16, tag="e")
                        for g in range(4):  # head groups of 4 -> one 2-bank psum
                            sc = psc.tile([128, 4 * QH], f32, tag="sc")
                            for hh in range(4):
                                h = g * 4 + hh
                                nc.tensor.matmul(
                                    sc[:, hh * QH : (hh + 1) * QH],
                                    KT[h // 2][:, kcs],
                                    QT[h][:, qsl],
                                    start=True,
                                    stop=True,
                                )
                            nc.scalar.activation(
                                e[:, g * 4 * QH : (g + 1) * 4 * QH],
                                sc[:],
                                AF.Exp,
                                scale=SCALE,
                            )
                        if attn_stop == "exp":
                            continue
                        # denominator over heads (tree), then 1/den
                        t1 = mid.tile([128, 8 * QH], bf16, tag="t1")
                        nc.vector.tensor_add(
                            t1[:], e[:, : 8 * QH], e[:, 8 * QH :]
                        )
                        t2 = mid.tile([128, 4 * QH], bf16, tag="t2")
                        nc.vector.tensor_add(
                            t2[:], t1[:, : 4 * QH], t1[:, 4 * QH :]
                        )
                        t3 = mid.tile([128, 2 * QH], bf16, tag="t3")
                        nc.vector.tensor_add(
                            t3[:], t2[:, : 2 * QH], t2[:, 2 * QH :]
                        )
                        den = mid.tile([128, QH], f32, tag="den")
                        nc.vector.tensor_add(den[:], t3[:, :QH], t3[:, QH:])
                        lden = mid.tile([128, QH], f32, tag="lden")
                        nc.scalar.activation(lden[:], den[:], AF.Ln)
                        r32 = mid.tile([128, QH], f32, tag="r32")
                        nc.scalar.activation(r32[:], lden[:], AF.Exp, scale=-1.0)
                        if attn_stop == "den":
                            continue
                        # broadcast recip to 4*QH then multiply per head-group
                        rb = mid.tile([128, 4 * QH], bf16, tag="rb")
                        nc.vector.tensor_copy(rb[:, :QH], r32[:])
                        nc.vector.tensor_copy(rb[:, QH : 2 * QH], rb[:, :QH])
                        nc.vector.tensor_copy(rb[:, 2 * QH :], rb[:, : 2 * QH])
                        w = wtsp.tile([128, H * QH], bf16, tag="w")
                        for g in range(4):
                            gs = slice(g * 4 * QH, (g + 1) * 4 * QH)
                            nc.vector.tensor_mul(w[:, gs], e[:, gs], rb[:])
                        if attn_stop == "mult":
                            continue
                        # AV: O^T[pair] += V_h^T-slice x w_h
                        for j in range(8):
                            cs = slice((j // 4) * QH, (j // 4 + 1) * QH)
                            for hh in range(2):
                                h = 2 * j + hh
                                # start=True clears the WHOLE 2KB psum bank
                                # row ("zero region"), so only the first pair
                                # in each bank (j<4) may start; j>=4 lands on
                                # already-pending-zero bytes.
                                nc.tensor.matmul(
                                    oacc[j % 4][hh * 64 : (hh + 1) * 64, cs],
                                    V[kc][:, h * 64 : (h + 1) * 64],
                                    w[:, h * QH : (h + 1) * QH],
                                    start=(kc == 0 and j < 4),
                                    stop=(kc == nkc - 1),
                                    skip_group_check=True,
                                )
                    if attn_stop == "full":
                        for j in range(8):
                            cs = slice((j // 4) * QH, (j // 4 + 1) * QH)
                            nc.scalar.copy(OT[j][:, qsl], oacc[j % 4][:, cs])

            # ---------------- Phase C: output projection -------------------
            if not phase_c:
                with tc.tile_pool(name="osb0", bufs=2) as osb0:
                    for q4 in range(4):
                        ob = osb0.tile([128, D], f32, tag="ob0")
                        nc.vector.tensor_copy(ob[:, 0:SQ], (OT if (do_attn and attn_stop == "full") else QT)[q4][:, :])
                        nc.vector.memset(ob[:, SQ:D], 0.0)
                        nc.gpsimd.dma_start(out_d[q4 * 128 : (q4 + 1) * 128, :], ob[:])
            else:
                with (
                    tc.tile_pool(name="pO", bufs=2, space="PSUM") as pO,
                    tc.tile_pool(name="wot", bufs=3) as wot,
                    tc.tile_pool(name="osb", bufs=2) as osb,
                ):
                    wo_t = [wot.tile([128, D], bf16, tag=f"wo{j}", name=f"wo{j}") for j in range(8)]
                    for j in range(8):
                        nc.sync.dma_start(wo_t[j][:], wo_d[j * 128 : (j + 1) * 128, :])
                    for q4 in range(4):
                        qsl = slice(q4 * 128, (q4 + 1) * 128)
                        po = pO.tile([128, D], f32, tag="po")
                        for j in range(8):
                            for f2 in range(2):
                                nc.tensor.matmul(
                                    po[:, f2 * 512 : (f2 + 1) * 512],
                                    OT[j][:, qsl],
                                    wo_t[j][:, f2 * 512 : (f2 + 1) * 512],
                                    start=(j == 0),
                                    stop=False,
                                )
                        for f2 in range(2):
                            nc.tensor.matmul(
                                po[:, f2 * 512 : (f2 + 1) * 512],
                                ones[0:1, 0:128],
                                bo_s[0:1, f2 * 512 : (f2 + 1) * 512],
                                start=False,
                                stop=True,
                            )
                        ob = osb.tile([128, D], f32, tag="ob")
                        nc.vector.tensor_copy(ob[:], po[:])
                        nc.gpsimd.dma_start(out_d[qsl, :], ob[:])

    _maybe_legalize(nc, legalize)
    return nc


def _maybe_legalize(nc, legalize):
    if legalize:
        _legalize_waits(nc)


def _prep_inputs(inputs):
    import ml_dtypes

    bf16 = ml_dtypes.bfloat16
    q = np.asarray(inputs["queries"], np.float32)
    k = np.asarray(inputs["keys"], np.float32)
    v = np.asarray(inputs["values"], np.float32)
    Wq = np.asarray(inputs["Wq"], np.float32).astype(bf16)
    Wk = np.asarray(inputs["Wk"], np.float32).astype(bf16)
    Wv = np.asarray(inputs["Wv"], np.float32).astype(bf16)
    Wo = np.asarray(inputs["Wo"], np.float32).astype(bf16)
    bq32 = np.asarray(inputs["bq"], np.float32)
    bk32 = np.asarray(inputs["bk"], np.float32)
    bqr = np.ascontiguousarray(bq32.reshape(8, 128).T)
    bkr = np.ascontiguousarray(bk32.reshape(8, 128).T)
    bq = bq32.astype(bf16).reshape(1, D)
    bk = np.asarray(inputs["bk"], np.float32).astype(bf16).reshape(1, D)
    bv = np.asarray(inputs["bv"], np.float32).astype(bf16).reshape(1, D)
    bo = np.asarray(inputs["bo"], np.float32).astype(bf16).reshape(1, D)

    kT = [np.ascontiguousarray(k[b].T).astype(bf16) for b in range(B)]
    vT = [np.ascontiguousarray(v[b].T).astype(bf16) for b in range(B)]

    in_maps = []
    for c in range(8):
        b, qq = c // 4, (c % 4) * SQ
        qT = np.ascontiguousarray(q[b, qq : qq + SQ, :].T).astype(bf16)
        in_maps.append(
            {
                "qT": qT,
                "kT": kT[b],
                "vT": vT[b],
                "wq": Wq,
                "wk": Wk,
                "wv": Wv,
                "wo": Wo,
                "bqr": bqr,
                "bkr": bkr,
                "bq": bq,
                "bk": bk,
                "bv": bv,
                "bo": bo,
            }
        )
    return in_maps


def run(inputs, trace=False, trace_kwargs=None):
    """Build (cached), run on 8 cores, return (output, BassKernelResults)."""
    from concourse.bass_utils import run_bass_kernel_spmd

    if "nc" not in _CACHE:
        _CACHE["nc"] = _build()
    nc = _CACHE["nc"]
    in_maps = _prep_inputs(inputs)
    res = run_bass_kernel_spmd(
        nc,
        in_maps,
        core_ids=list(range(8)),
        trace=trace,
        **(trace_kwargs or {}),
    )
    out = np.empty((B, S, D), np.float32)
    for c in range(8):
        b, qq = c // 4, (c % 4) * SQ
        out[b, qq : qq + SQ, :] = res.results[c]["out"]
    return out, res


def kernel(**inputs) -> np.ndarray:
    out, _ = run(inputs, trace=False)
    return out



# revision 15
# speedup vs baseline: 1.0451x; 1.0451x over previous
"""Trainium2 Bass kernel for nn_MultiHeadAttention_22883585753377.

Reference semantics (torch legacy): softmax over the HEADS axis (dim=1) of
the [B,H,S,S] score tensor, scale = sqrt(KEY_DIM)=32.

Sharding: 8 cores = (batch b, query-quarter). Each core handles b = c//4 and
512 query rows, all 16 heads (the heads-softmax couples heads, so they stay
local).

v2: single fused pass. K/V projections are produced just-in-time inside the
first query-half's attention loop so their matmuls overlap the exp (scalar)
and softmax-tree/multiply (vector) work of earlier k-chunks. Scores use one
matmul per head-PAIR (stationary = K^T pair chunk [128 feat, 128 k], moving =
zero-padded Q pair tile [128, 512]) instead of two per head. PSUM: 4 banks of
AV accumulators + 2 rotating [128,1024] tiles shared by scores, projections,
and the output projection.
"""

import numpy as np

B = 2
S = 1024 * 2
D = 1024
H = 16
DH = 64
SQ = 512  # query rows per core
QH = 256  # q processed per half
KC = 128  # k-chunk (partition dim of scores^T tiles)
NKC = S // KC  # 16
KG = 4  # k-chunks per jit projection group
NG = NKC // KG  # 4 groups of 512 k/s rows
SCALE = 1.0 / 32.0  # 1/sqrt(KEY_DIM)

_CACHE = {}


def _legalize_waits(nc):
    """This container's walrus encodes at most ONE semaphore wait per
    instruction; Tile emits up to ~10. Split the excess onto same-engine nops
    inserted immediately before the instruction."""
    import bass_rust

    ctr = [0]
    for bb in nc.main_func.blocks:
        insts = list(bb.instructions)
        out = []
        changed = False
        for ins in insts:
            si = ins.sync_info
            waits = list(si.on_wait) if si is not None and si.on_wait else []
            if len(waits) > 1:
                changed = True
                upd = list(si.on_update) if si.on_update else []
                for w in waits[:-1]:
                    ctr[0] += 1
                    nop = bass_rust.InstNoOp(
                        name=f"I-wsplit-{ctr[0]}", ins=[], outs=[]
                    )
                    nop.engine = ins.engine
                    nop.bass_nofuse = True
                    nop.sync_info = bass_rust.SyncInfo(on_wait=[w], on_update=[])
                    out.append(nop)
                ins.sync_info = bass_rust.SyncInfo(
                    on_wait=[waits[-1]], on_update=upd
                )
            out.append(ins)
        if changed:
            bb.instructions = out


def _build(legalize=True):
    import concourse.bass as bass
    import concourse.mybir as mybir
    import concourse.tile as tile

    bf16 = mybir.dt.bfloat16
    f32 = mybir.dt.float32
    AF = mybir.ActivationFunctionType

    nc = bass.Bass()

    # --- I/O ---------------------------------------------------------------
    qT_d = nc.dram_tensor("qT", [D, SQ], bf16, kind="ExternalInput")
    kT_d = nc.dram_tensor("kT", [D, S], bf16, kind="ExternalInput")
    vT_d = nc.dram_tensor("vT", [D, S], bf16, kind="ExternalInput")
    wq_d = nc.dram_tensor("wq", [D, D], bf16, kind="ExternalInput")
    wk_d = nc.dram_tensor("wk", [D, D], bf16, kind="ExternalInput")
    wv_d = nc.dram_tensor("wv", [D, D], bf16, kind="ExternalInput")
    wo_d = nc.dram_tensor("wo", [D, D], bf16, kind="ExternalInput")
    bqr_d = nc.dram_tensor("bqr", [128, 8], f32, kind="ExternalInput")
    bkr_d = nc.dram_tensor("bkr", [128, 8], f32, kind="ExternalInput")
    bv_d = nc.dram_tensor("bv", [1, D], bf16, kind="ExternalInput")
    bo_d = nc.dram_tensor("bo", [1, D], bf16, kind="ExternalInput")
    out_d = nc.dram_tensor("out", [SQ, D], f32, kind="ExternalOutput")

    with tile.TileContext(nc) as tc:
        with (
            tc.tile_pool(name="persist", bufs=1) as persist,
            tc.tile_pool(name="consts", bufs=1) as consts,
            tc.tile_pool(name="wkv", bufs=1) as wkv,
            tc.tile_pool(name="kraw", bufs=1) as krawp,
            tc.tile_pool(name="vsl", bufs=1) as vsl,
            tc.tile_pool(name="ps", bufs=2, space="PSUM") as psp,
            tc.tile_pool(name="pav", bufs=1, space="PSUM") as pav,
            tc.tile_pool(name="exp", bufs=2) as expp,
            tc.tile_pool(name="wts", bufs=6) as wtsp,
            tc.tile_pool(name="midA", bufs=1) as midA,
            tc.tile_pool(name="midB", bufs=2) as midB,
            tc.tile_pool(name="wrow", bufs=1) as wrow,
        ):
            # ---- persistent SBUF ------------------------------------------
            # K^T [D,S]: pair chunk f holds heads 2f (rows 0-63), 2f+1
            # (rows 64-127); filled group by group.
            KT = [persist.tile([128, S], bf16, tag=f"KT{f}", name=f"KT{f}") for f in range(8)]
            # V natural [S,D] as 16 x [128, D]
            V = [persist.tile([128, D], bf16, tag=f"V{s}", name=f"V{s}") for s in range(16)]
            # Q^T pair tiles [128, 2*SQ]: cols [qh0: even 256 | odd 256]
            # [qh1: even | odd]; even head rows 0-63 (64-127 zero), odd head
            # rows 64-127 (0-63 zero).
            QTP = [persist.tile([128, 2 * SQ], bf16, tag=f"QTP{f}", name=f"QTP{f}") for f in range(8)]
            # O^T [D,SQ] as 8 x [128, SQ] (pair j = heads 2j,2j+1)
            OT = [persist.tile([128, SQ], bf16, tag=f"OT{p}", name=f"OT{p}") for p in range(8)]

            ones = consts.tile([1, 128], bf16)
            nc.vector.memset(ones[:], 1.0)
            bqr_s = consts.tile([128, 8], f32, tag="bqr")
            bkr_s = consts.tile([128, 8], f32, tag="bkr")
            nc.sync.dma_start(bqr_s[:], bqr_d[:])
            nc.sync.dma_start(bkr_s[:], bkr_d[:])
            bv_s = consts.tile([1, D], bf16, tag="bv")
            bo_s = consts.tile([1, D], bf16, tag="bo")
            nc.sync.dma_start(bv_s[:], bv_d[:])
            nc.sync.dma_start(bo_s[:], bo_d[:])

            wkr = [wkv.tile([128, D], bf16, tag=f"wkr{d}", name=f"wkr{d}") for d in range(8)]
            wvr = [wkv.tile([128, D], bf16, tag=f"wvr{d}", name=f"wvr{d}") for d in range(8)]
            for d in range(8):
                nc.sync.dma_start(wkr[d][:], wk_d[d * 128 : (d + 1) * 128, :])
                nc.sync.dma_start(wvr[d][:], wv_d[d * 128 : (d + 1) * 128, :])

            def load_kraw(g):
                """kraw group g: 8 tiles [128, 512] (cols g*512..)."""
                ts = []
                for d in range(8):
                    t = krawp.tile([128, 512], bf16, tag=f"kraw{d}", name=f"kraw{d}g{g}")
                    nc.sync.dma_start(t[:], kT_d[d * 128 : (d + 1) * 128, g * 512 : (g + 1) * 512])
                    ts.append(t)
                return ts

            def load_vsl(sc):
                """vT slice for s-chunk sc: 8 tiles [128, 128]."""
                ts = []
                for d in range(8):
                    t = vsl.tile([128, 128], bf16, tag=f"vsl{d}", name=f"vsl{d}s{sc}")
                    nc.sync.dma_start(t[:], vT_d[d * 128 : (d + 1) * 128, sc * 128 : (sc + 1) * 128])
                    ts.append(t)
                return ts

            def proj_k(g, fo_list, kr):
                """Project K chunks for group g, pair-features fo in fo_list."""
                bt = psp.tile([128, 512 * len(fo_list)], f32, tag="ps")
                for i, fo in enumerate(fo_list):
                    for d in range(8):
                        nc.tensor.matmul(
                            bt[:, i * 512 : (i + 1) * 512],
                            wkr[d][:, fo * 128 : (fo + 1) * 128],
                            kr[d][:],
                            start=(d == 0),
                            stop=(d == 7),
                        )
                for i, fo in enumerate(fo_list):
                    nc.scalar.activation(
                        KT[fo][:, g * 512 : (g + 1) * 512],
                        bt[:, i * 512 : (i + 1) * 512],
                        AF.Identity,
                        bias=bkr_s[:, fo : fo + 1],
                    )

            def proj_v(sc, vs):
                """Project V s-chunk sc (V[sc] = [128, D])."""
                pv = psp.tile([128, D], f32, tag="ps")
                for f2 in range(2):
                    fs = slice(f2 * 512, (f2 + 1) * 512)
                    for d in range(8):
                        nc.tensor.matmul(
                            pv[:, fs],
                            vs[d][:],
                            wvr[d][:, fs],
                            start=(d == 0),
                            stop=False,
                        )
                    nc.tensor.matmul(
                        pv[:, fs], ones[0:1, :], bv_s[0:1, fs], start=False, stop=True
                    )
                nc.scalar.copy(V[sc][:], pv[:])

            # ---- prologue: Q proj + K/V group 0 ---------------------------
            with tc.tile_pool(name="qraw", bufs=1) as qrawp:
                for f in range(8):
                    nc.vector.memset(QTP[f][:], 0.0)
                qraw = [qrawp.tile([128, SQ], bf16, tag=f"qraw{d}", name=f"qraw{d}") for d in range(8)]
                wqr = [wrow.tile([128, D], bf16, tag=f"w{d}", name=f"wqr{d}") for d in range(8)]
                for d in range(8):
                    nc.sync.dma_start(qraw[d][:], qT_d[d * 128 : (d + 1) * 128, :])
                    nc.sync.dma_start(wqr[d][:], wq_d[d * 128 : (d + 1) * 128, :])

                kr0 = load_kraw(0)
                vs_pend = load_vsl(0)

                for fp in range(4):  # feature pair-chunks: f = 2fp, 2fp+1
                    ps = psp.tile([128, 1024], f32, tag="ps")
                    for i in range(2):
                        f = 2 * fp + i
                        for d in range(8):
                            nc.tensor.matmul(
                                ps[:, i * 512 : (i + 1) * 512],
                                wqr[d][:, f * 128 : (f + 1) * 128],
                                qraw[d][:],
                                start=(d == 0),
                                stop=(d == 7),
                            )
                    for i in range(2):
                        f = 2 * fp + i
                        for qh in range(2):
                            qs = slice(i * 512 + qh * 256, i * 512 + (qh + 1) * 256)
                            nc.scalar.activation(
                                QTP[f][0:64, qh * 512 : qh * 512 + 256],
                                ps[0:64, qs],
                                AF.Identity,
                                bias=bqr_s[0:64, f : f + 1],
                            )
                            nc.scalar.activation(
                                QTP[f][64:128, qh * 512 + 256 : (qh + 1) * 512],
                                ps[64:128, qs],
                                AF.Identity,
                                bias=bqr_s[64:128, f : f + 1],
                            )

                # K group 0: 8 pair-features in 4 borrowed tiles
                for fp in range(4):
                    proj_k(0, [2 * fp, 2 * fp + 1], kr0)
                # V group 0 (s-chunks 0-3)
                for sc in range(4):
                    vs = vs_pend
                    if sc < 3:
                        vs_pend = load_vsl(sc + 1)
                    proj_v(sc, vs)

            # ---- attention ------------------------------------------------
            kr_pend = load_kraw(1)
            for qh in range(2):
                qsl = slice(qh * QH, (qh + 1) * QH)
                oacc = [
                    pav.tile([128, 2 * QH], f32, tag=f"oacc{i}", name=f"oacc{i}q{qh}", bufs=1)
                    for i in range(4)
                ]
                for kc in range(NKC):
                    # scores + exp: 2 pairs per rotating psum tile
                    kcs = slice(kc * 128, (kc + 1) * 128)
                    e = expp.tile([128, H * QH], bf16, tag="e")
                    for g2 in range(4):
                        sc2 = psp.tile([128, 1024], f32, tag="ps")
                        for i in range(2):
                            f = 2 * g2 + i
                            nc.tensor.matmul(
                                sc2[:, i * 512 : (i + 1) * 512],
                                KT[f][:, kcs],
                                QTP[f][:, qh * 512 : (qh + 1) * 512],
                                start=True,
                                stop=True,
                            )
                        nc.scalar.activation(
                            e[:, g2 * 1024 : (g2 + 1) * 1024],
                            sc2[:],
                            AF.Exp,
                            scale=SCALE,
                        )
                    # JIT K/V projection for group kc//4 + 1 (qh0 only)
                    if qh == 0 and kc < 12:
                        g = kc // 4 + 1
                        s = kc % 4
                        if s == 0:
                            kr_cur = kr_pend
                        proj_k(g, [2 * s, 2 * s + 1], kr_cur)
                        vs = vs_pend
                        if not (g == 3 and s == 3):
                            vs_pend = load_vsl(4 * g + s + 1)
                        proj_v(4 * g + s, vs)
                        if s == 3 and g < 3:
                            kr_pend = load_kraw(g + 1)

                    # denominator over heads (tree), then 1/den
                    t1 = midA.tile([128, 8 * QH], bf16, tag="t1")
                    nc.vector.tensor_add(t1[:], e[:, : 8 * QH], e[:, 8 * QH :])
                    t2 = midA.tile([128, 4 * QH], bf16, tag="t2")
                    nc.vector.tensor_add(t2[:], t1[:, : 4 * QH], t1[:, 4 * QH :])
                    t3 = midA.tile([128, 2 * QH], bf16, tag="t3")
                    nc.vector.tensor_add(t3[:], t2[:, : 2 * QH], t2[:, 2 * QH :])
                    den = midB.tile([128, QH], bf16, tag="den")
                    nc.vector.tensor_add(den[:], t3[:, :QH], t3[:, QH:])
                    lden = midB.tile([128, QH], f32, tag="lden")
                    nc.scalar.activation(lden[:], den[:], AF.Ln)
                    rb = midB.tile([128, 4 * QH], bf16, tag="rb")
                    nc.scalar.activation(rb[:, :QH], lden[:], AF.Exp, scale=-1.0)
                    nc.vector.tensor_copy(rb[:, QH : 2 * QH], rb[:, :QH])
                    nc.vector.tensor_copy(rb[:, 2 * QH :], rb[:, : 2 * QH])
                    # weights = e * (1/den), per 2-pair chunk
                    wch = []
                    for g2 in range(4):
                        gs = slice(g2 * 4 * QH, (g2 + 1) * 4 * QH)
                        w = wtsp.tile([128, 4 * QH], bf16, tag="w")
                        nc.vector.tensor_mul(w[:], e[:, gs], rb[:])
                        wch.append(w)
                    # AV: O^T[pair] += V_h^T-slice x w_h
                    for j in range(8):
                        cs = slice((j // 4) * QH, (j // 4 + 1) * QH)
                        for hh in range(2):
                            h = 2 * j + hh
                            nc.tensor.matmul(
                                oacc[j % 4][hh * 64 : (hh + 1) * 64, cs],
                                V[kc][:, h * 64 : (h + 1) * 64],
                                wch[j // 2][:, (j % 2) * 512 + hh * 256 : (j % 2) * 512 + (hh + 1) * 256],
                                start=(kc == 0 and j < 4),
                                stop=(kc == NKC - 1),
                                skip_group_check=True,
                            )
                for j in range(8):
                    cs = slice((j // 4) * QH, (j // 4 + 1) * QH)
                    nc.scalar.copy(OT[j][:, qsl], oacc[j % 4][:, cs])

            # ---- output projection ----------------------------------------
            with tc.tile_pool(name="osb", bufs=2) as osb:
                wo_t = [wrow.tile([128, D], bf16, tag=f"w{j}", name=f"wo{j}") for j in range(8)]
                for j in range(8):
                    nc.sync.dma_start(wo_t[j][:], wo_d[j * 128 : (j + 1) * 128, :])
                for q4 in range(4):
                    qsl = slice(q4 * 128, (q4 + 1) * 128)
                    po = psp.tile([128, D], f32, tag="ps")
                    for f2 in range(2):
                        fs = slice(f2 * 512, (f2 + 1) * 512)
                        for j in range(8):
                            nc.tensor.matmul(
                                po[:, fs],
                                OT[j][:, qsl],
                                wo_t[j][:, fs],
                                start=(j == 0),
                                stop=False,
                            )
                        nc.tensor.matmul(
                            po[:, fs], ones[0:1, :], bo_s[0:1, fs], start=False, stop=True
                        )
                    ob = osb.tile([128, D], f32, tag="ob")
                    nc.vector.tensor_copy(ob[:], po[:])
                    nc.gpsimd.dma_start(out_d[qsl, :], ob[:])

    if legalize:
        _legalize_waits(nc)
    return nc


def _prep_inputs(inputs):
    import ml_dtypes

    bf16 = ml_dtypes.bfloat16
    q = np.asarray(inputs["queries"], np.float32)
    k = np.asarray(inputs["keys"], np.float32)
    v = np.asarray(inputs["values"], np.float32)
    Wq = np.asarray(inputs["Wq"], np.float32).astype(bf16)
    Wk = np.asarray(inputs["Wk"], np.float32).astype(bf16)
    Wv = np.asarray(inputs["Wv"], np.float32).astype(bf16)
    Wo = np.asarray(inputs["Wo"], np.float32).astype(bf16)
    bq32 = np.asarray(inputs["bq"], np.float32)
    bk32 = np.asarray(inputs["bk"], np.float32)
    bqr = np.ascontiguousarray(bq32.reshape(8, 128).T)
    bkr = np.ascontiguousarray(bk32.reshape(8, 128).T)
    bv = np.asarray(inputs["bv"], np.float32).astype(bf16).reshape(1, D)
    bo = np.asarray(inputs["bo"], np.float32).astype(bf16).reshape(1, D)

    kT = [np.ascontiguousarray(k[b].T).astype(bf16) for b in range(B)]
    vT = [np.ascontiguousarray(v[b].T).astype(bf16) for b in range(B)]

    in_maps = []
    for c in range(8):
        b, qq = c // 4, (c % 4) * SQ
        qT = np.ascontiguousarray(q[b, qq : qq + SQ, :].T).astype(bf16)
        in_maps.append(
            {
                "qT": qT,
                "kT": kT[b],
                "vT": vT[b],
                "wq": Wq,
                "wk": Wk,
                "wv": Wv,
                "wo": Wo,
                "bqr": bqr,
                "bkr": bkr,
                "bv": bv,
                "bo": bo,
            }
        )
    return in_maps


def run(inputs, trace=False, trace_kwargs=None):
    """Build (cached), run on 8 cores, return (output, BassKernelResults)."""
    from concourse.bass_utils import run_bass_kernel_spmd

    if "nc" not in _CACHE:
        _CACHE["nc"] = _build()
    nc = _CACHE["nc"]
    in_maps = _prep_inputs(inputs)
    res = run_bass_kernel_spmd(
        nc,
        in_maps,
        core_ids=list(range(8)),
        trace=trace,
        **(trace_kwargs or {}),
    )
    out = np.empty((B, S, D), np.float32)
    for c in range(8):
        b, qq = c // 4, (c % 4) * SQ
        out[b, qq : qq + SQ, :] = res.results[c]["out"]
    return out, res


def kernel(**inputs) -> np.ndarray:
    out, _ = run(inputs, trace=False)
    return out


# revision 16
# speedup vs baseline: 1.1668x; 1.1165x over previous
"""Trainium2 Bass kernel for nn_MultiHeadAttention_22883585753377.

Reference semantics (torch legacy): softmax over the HEADS axis (dim=1) of
the [B,H,S,S] score tensor, scale = sqrt(KEY_DIM)=32.

Sharding: 8 cores = (batch b, query-quarter). Each core handles b = c//4 and
512 query rows, all 16 heads (the heads-softmax couples heads, so they stay
local).

v2: single fused pass. K/V projections are produced just-in-time inside the
first query-half's attention loop so their matmuls overlap the exp (scalar)
and softmax-tree/multiply (vector) work of earlier k-chunks. Scores use one
matmul per head-PAIR (stationary = K^T pair chunk [128 feat, 128 k], moving =
zero-padded Q pair tile [128, 512]) instead of two per head. PSUM: 4 banks of
AV accumulators + 2 rotating [128,1024] tiles shared by scores, projections,
and the output projection.
"""

import numpy as np

B = 2
S = 1024 * 2
D = 1024
H = 16
DH = 64
SQ = 512  # query rows per core
QH = 256  # q processed per half
KC = 128  # k-chunk (partition dim of scores^T tiles)
NKC = S // KC  # 16
KG = 4  # k-chunks per jit projection group
NG = NKC // KG  # 4 groups of 512 k/s rows
SCALE = 1.0 / 32.0  # 1/sqrt(KEY_DIM)

_CACHE = {}


def _legalize_waits(nc):
    """This container's walrus encodes at most ONE semaphore wait per
    instruction; Tile emits up to ~10. Split the excess onto same-engine nops
    inserted immediately before the instruction."""
    import bass_rust

    ctr = [0]
    for bb in nc.main_func.blocks:
        insts = list(bb.instructions)
        out = []
        changed = False
        for ins in insts:
            si = ins.sync_info
            waits = list(si.on_wait) if si is not None and si.on_wait else []
            if len(waits) > 1:
                changed = True
                upd = list(si.on_update) if si.on_update else []
                for w in waits[:-1]:
                    ctr[0] += 1
                    nop = bass_rust.InstNoOp(
                        name=f"I-wsplit-{ctr[0]}", ins=[], outs=[]
                    )
                    nop.engine = ins.engine
                    nop.bass_nofuse = True
                    nop.sync_info = bass_rust.SyncInfo(on_wait=[w], on_update=[])
                    out.append(nop)
                ins.sync_info = bass_rust.SyncInfo(
                    on_wait=[waits[-1]], on_update=upd
                )
            out.append(ins)
        if changed:
            bb.instructions = out


def _build(legalize=True):
    import concourse.bass as bass
    import concourse.mybir as mybir
    import concourse.tile as tile

    bf16 = mybir.dt.bfloat16
    f32 = mybir.dt.float32
    AF = mybir.ActivationFunctionType

    nc = bass.Bass()

    # --- I/O ---------------------------------------------------------------
    qT_d = nc.dram_tensor("qT", [D, SQ], bf16, kind="ExternalInput")
    kT_d = nc.dram_tensor("kT", [D, S], bf16, kind="ExternalInput")
    vT_d = nc.dram_tensor("vT", [D, S], bf16, kind="ExternalInput")
    wq_d = nc.dram_tensor("wq", [D, D], bf16, kind="ExternalInput")
    wk_d = nc.dram_tensor("wk", [D, D], bf16, kind="ExternalInput")
    wv_d = nc.dram_tensor("wv", [D, D], bf16, kind="ExternalInput")
    wo_d = nc.dram_tensor("wo", [D, D], bf16, kind="ExternalInput")
    bqr_d = nc.dram_tensor("bqr", [128, 8], f32, kind="ExternalInput")
    bkr_d = nc.dram_tensor("bkr", [128, 8], f32, kind="ExternalInput")
    bv_d = nc.dram_tensor("bv", [1, D], bf16, kind="ExternalInput")
    bo_d = nc.dram_tensor("bo", [1, D], bf16, kind="ExternalInput")
    out_d = nc.dram_tensor("out", [SQ, D], f32, kind="ExternalOutput")

    with tile.TileContext(nc) as tc:
        with (
            tc.tile_pool(name="persist", bufs=1) as persist,
            tc.tile_pool(name="consts", bufs=1) as consts,
            tc.tile_pool(name="wkv", bufs=1) as wkv,
            tc.tile_pool(name="kraw", bufs=1) as krawp,
            tc.tile_pool(name="vsl", bufs=1) as vsl,
            tc.tile_pool(name="ps", bufs=2, space="PSUM") as psp,
            tc.tile_pool(name="pav", bufs=1, space="PSUM") as pav,
            tc.tile_pool(name="exp", bufs=2) as expp,
            tc.tile_pool(name="wts", bufs=5) as wtsp,
            tc.tile_pool(name="midA", bufs=1) as midA,
            tc.tile_pool(name="midB", bufs=2) as midB,
            tc.tile_pool(name="wrow", bufs=1) as wrow,
            tc.tile_pool(name="osb", bufs=1) as osbp,
        ):
            # ---- persistent SBUF ------------------------------------------
            # K^T [D,S]: pair chunk f holds heads 2f (rows 0-63), 2f+1
            # (rows 64-127); filled group by group.
            KT = [persist.tile([128, S], bf16, tag=f"KT{f}", name=f"KT{f}") for f in range(8)]
            # V natural [S,D] as 16 x [128, D]
            V = [persist.tile([128, D], bf16, tag=f"V{s}", name=f"V{s}") for s in range(16)]
            # Q^T pair tiles [128, 2*SQ]: cols [qh0: even 256 | odd 256]
            # [qh1: even | odd]; even head rows 0-63 (64-127 zero), odd head
            # rows 64-127 (0-63 zero).
            QTP = [persist.tile([128, 2 * SQ], bf16, tag=f"QTP{f}", name=f"QTP{f}") for f in range(8)]
            # O^T [D,SQ] as 8 x [128, SQ] (pair j = heads 2j,2j+1)
            OT = [persist.tile([128, SQ], bf16, tag=f"OT{p}", name=f"OT{p}") for p in range(8)]

            ones = consts.tile([1, 128], bf16)
            nc.vector.memset(ones[:], 1.0)
            bqr_s = consts.tile([128, 8], f32, tag="bqr")
            bkr_s = consts.tile([128, 8], f32, tag="bkr")
            nc.sync.dma_start(bqr_s[:], bqr_d[:])
            nc.sync.dma_start(bkr_s[:], bkr_d[:])
            bv_s = consts.tile([1, D], bf16, tag="bv")
            bo_s = consts.tile([1, D], bf16, tag="bo")
            nc.sync.dma_start(bv_s[:], bv_d[:])
            nc.sync.dma_start(bo_s[:], bo_d[:])

            wkr = [wkv.tile([128, D], bf16, tag=f"wkr{d}", name=f"wkr{d}") for d in range(8)]
            wvr = [wkv.tile([128, D], bf16, tag=f"wvr{d}", name=f"wvr{d}") for d in range(8)]
            for d in range(8):
                nc.sync.dma_start(wkr[d][:], wk_d[d * 128 : (d + 1) * 128, :])
                nc.sync.dma_start(wvr[d][:], wv_d[d * 128 : (d + 1) * 128, :])

            def load_kraw(g):
                """kraw group g: 8 tiles [128, 512] (cols g*512..)."""
                ts = []
                for d in range(8):
                    t = krawp.tile([128, 512], bf16, tag=f"kraw{d}", name=f"kraw{d}g{g}")
                    nc.sync.dma_start(t[:], kT_d[d * 128 : (d + 1) * 128, g * 512 : (g + 1) * 512])
                    ts.append(t)
                return ts

            def load_vsl(sc):
                """vT slice for s-chunk sc: 8 tiles [128, 128]."""
                ts = []
                for d in range(8):
                    t = vsl.tile([128, 128], bf16, tag=f"vsl{d}", name=f"vsl{d}s{sc}")
                    nc.sync.dma_start(t[:], vT_d[d * 128 : (d + 1) * 128, sc * 128 : (sc + 1) * 128])
                    ts.append(t)
                return ts

            def proj_k(g, fo_list, kr):
                """Project K chunks for group g, pair-features fo in fo_list."""
                bt = psp.tile([128, 512 * len(fo_list)], f32, tag="ps")
                for i, fo in enumerate(fo_list):
                    for d in range(8):
                        nc.tensor.matmul(
                            bt[:, i * 512 : (i + 1) * 512],
                            wkr[d][:, fo * 128 : (fo + 1) * 128],
                            kr[d][:],
                            start=(d == 0),
                            stop=(d == 7),
                        )
                for i, fo in enumerate(fo_list):
                    nc.scalar.activation(
                        KT[fo][:, g * 512 : (g + 1) * 512],
                        bt[:, i * 512 : (i + 1) * 512],
                        AF.Identity,
                        bias=bkr_s[:, fo : fo + 1],
                    )

            def proj_v(sc, vs):
                """Project V s-chunk sc (V[sc] = [128, D])."""
                pv = psp.tile([128, D], f32, tag="ps")
                for f2 in range(2):
                    fs = slice(f2 * 512, (f2 + 1) * 512)
                    for d in range(8):
                        nc.tensor.matmul(
                            pv[:, fs],
                            vs[d][:],
                            wvr[d][:, fs],
                            start=(d == 0),
                            stop=False,
                        )
                    nc.tensor.matmul(
                        pv[:, fs], ones[0:1, :], bv_s[0:1, fs], start=False, stop=True
                    )
                nc.scalar.copy(V[sc][:], pv[:])

            # ---- prologue: Q proj + K/V group 0 ---------------------------
            with tc.tile_pool(name="qraw", bufs=1) as qrawp:
                for f in range(8):
                    nc.vector.memset(QTP[f][:], 0.0)
                qraw = [qrawp.tile([128, SQ], bf16, tag=f"qraw{d}", name=f"qraw{d}") for d in range(8)]
                wqr = [wrow.tile([128, D], bf16, tag=f"w{d}", name=f"wqr{d}") for d in range(8)]
                for d in range(8):
                    nc.sync.dma_start(qraw[d][:], qT_d[d * 128 : (d + 1) * 128, :])
                    nc.sync.dma_start(wqr[d][:], wq_d[d * 128 : (d + 1) * 128, :])

                kr0 = load_kraw(0)
                vs_pend = load_vsl(0)

                for fp in range(4):  # feature pair-chunks: f = 2fp, 2fp+1
                    ps = psp.tile([128, 1024], f32, tag="ps")
                    for i in range(2):
                        f = 2 * fp + i
                        for d in range(8):
                            nc.tensor.matmul(
                                ps[:, i * 512 : (i + 1) * 512],
                                wqr[d][:, f * 128 : (f + 1) * 128],
                                qraw[d][:],
                                start=(d == 0),
                                stop=(d == 7),
                            )
                    for i in range(2):
                        f = 2 * fp + i
                        for qh in range(2):
                            qs = slice(i * 512 + qh * 256, i * 512 + (qh + 1) * 256)
                            nc.scalar.activation(
                                QTP[f][0:64, qh * 512 : qh * 512 + 256],
                                ps[0:64, qs],
                                AF.Identity,
                                bias=bqr_s[0:64, f : f + 1],
                            )
                            nc.scalar.activation(
                                QTP[f][64:128, qh * 512 + 256 : (qh + 1) * 512],
                                ps[64:128, qs],
                                AF.Identity,
                                bias=bqr_s[64:128, f : f + 1],
                            )

                # K group 0: 8 pair-features in 4 borrowed tiles
                for fp in range(4):
                    proj_k(0, [2 * fp, 2 * fp + 1], kr0)
                # V group 0 (s-chunks 0-3)
                for sc in range(4):
                    vs = vs_pend
                    if sc < 3:
                        vs_pend = load_vsl(sc + 1)
                    proj_v(sc, vs)

            # ---- attention ------------------------------------------------
            kr_pend = load_kraw(1)
            wo_t = [wrow.tile([128, D], bf16, tag=f"w{j}", name=f"wo{j}") for j in range(8)]
            for j in range(8):
                nc.sync.dma_start(wo_t[j][:], wo_d[j * 128 : (j + 1) * 128, :])

            def oproj(q4, osb):
                qsl = slice(q4 * 128, (q4 + 1) * 128)
                po = psp.tile([128, D], f32, tag="ps")
                for f2 in range(2):
                    fs = slice(f2 * 512, (f2 + 1) * 512)
                    for j in range(8):
                        nc.tensor.matmul(
                            po[:, fs],
                            OT[j][:, qsl],
                            wo_t[j][:, fs],
                            start=(j == 0),
                            stop=False,
                        )
                    nc.tensor.matmul(
                        po[:, fs], ones[0:1, :], bo_s[0:1, fs], start=False, stop=True
                    )
                ob = osb.tile([128, D], f32, tag="ob")
                nc.vector.tensor_copy(ob[:], po[:])
                nc.gpsimd.dma_start(out_d[qsl, :], ob[:])

            for qh in range(2):
                qsl = slice(qh * QH, (qh + 1) * QH)
                oacc = [
                    pav.tile([128, 2 * QH], f32, tag=f"oacc{i}", name=f"oacc{i}q{qh}", bufs=1)
                    for i in range(4)
                ]
                for kc in range(NKC):
                    # scores + exp: 2 pairs per rotating psum tile
                    kcs = slice(kc * 128, (kc + 1) * 128)
                    e = expp.tile([128, H * QH], bf16, tag="e")
                    for g2 in range(4):
                        sc2 = psp.tile([128, 1024], f32, tag="ps")
                        for i in range(2):
                            f = 2 * g2 + i
                            nc.tensor.matmul(
                                sc2[:, i * 512 : (i + 1) * 512],
                                KT[f][:, kcs],
                                QTP[f][:, qh * 512 : (qh + 1) * 512],
                                start=True,
                                stop=True,
                            )
                        nc.scalar.activation(
                            e[:, g2 * 1024 : (g2 + 1) * 1024],
                            sc2[:],
                            AF.Exp,
                            scale=SCALE,
                        )
                    # JIT K/V projection for group kc//4 + 1 (qh0 only)
                    if qh == 0 and kc < 12:
                        g = kc // 4 + 1
                        s = kc % 4
                        if s == 0:
                            kr_cur = kr_pend
                        proj_k(g, [2 * s, 2 * s + 1], kr_cur)
                        vs = vs_pend
                        if not (g == 3 and s == 3):
                            vs_pend = load_vsl(4 * g + s + 1)
                        proj_v(4 * g + s, vs)
                        if s == 3 and g < 3:
                            kr_pend = load_kraw(g + 1)

                    # denominator over heads (tree), then 1/den
                    t1 = midA.tile([128, 8 * QH], bf16, tag="t1")
                    nc.vector.tensor_add(t1[:], e[:, : 8 * QH], e[:, 8 * QH :])
                    t2 = midA.tile([128, 4 * QH], bf16, tag="t2")
                    nc.vector.tensor_add(t2[:], t1[:, : 4 * QH], t1[:, 4 * QH :])
                    t3 = midA.tile([128, 2 * QH], bf16, tag="t3")
                    nc.vector.tensor_add(t3[:], t2[:, : 2 * QH], t2[:, 2 * QH :])
                    den = midB.tile([128, QH], bf16, tag="den")
                    nc.vector.tensor_add(den[:], t3[:, :QH], t3[:, QH:])
                    lden = midB.tile([128, QH], f32, tag="lden")
                    nc.scalar.activation(lden[:], den[:], AF.Ln)
                    rb = midB.tile([128, 4 * QH], bf16, tag="rb")
                    nc.scalar.activation(rb[:, :QH], lden[:], AF.Exp, scale=-1.0)
                    nc.vector.tensor_copy(rb[:, QH : 2 * QH], rb[:, :QH])
                    nc.vector.tensor_copy(rb[:, 2 * QH :], rb[:, : 2 * QH])
                    # weights = e * (1/den), per 2-pair chunk
                    wch = []
                    for g2 in range(4):
                        gs = slice(g2 * 4 * QH, (g2 + 1) * 4 * QH)
                        w = wtsp.tile([128, 4 * QH], bf16, tag="w")
                        nc.vector.tensor_mul(w[:], e[:, gs], rb[:])
                        wch.append(w)
                    # AV: O^T[pair] += V_h^T-slice x w_h
                    for j in range(8):
                        cs = slice((j // 4) * QH, (j // 4 + 1) * QH)
                        for hh in range(2):
                            h = 2 * j + hh
                            nc.tensor.matmul(
                                oacc[j % 4][hh * 64 : (hh + 1) * 64, cs],
                                V[kc][:, h * 64 : (h + 1) * 64],
                                wch[j // 2][:, (j % 2) * 512 + hh * 256 : (j % 2) * 512 + (hh + 1) * 256],
                                start=(kc == 0 and j < 4),
                                stop=(kc == NKC - 1),
                                skip_group_check=True,
                            )
                for j in range(8):
                    cs = slice((j // 4) * QH, (j // 4 + 1) * QH)
                    nc.scalar.copy(OT[j][:, qsl], oacc[j % 4][:, cs])
                oproj(2 * qh, osbp)
                oproj(2 * qh + 1, osbp)


    if legalize:
        _legalize_waits(nc)
    return nc


def _prep_inputs(inputs):
    import ml_dtypes

    bf16 = ml_dtypes.bfloat16
    q = np.asarray(inputs["queries"], np.float32)
    k = np.asarray(inputs["keys"], np.float32)
    v = np.asarray(inputs["values"], np.float32)
    Wq = np.asarray(inputs["Wq"], np.float32).astype(bf16)
    Wk = np.asarray(inputs["Wk"], np.float32).astype(bf16)
    Wv = np.asarray(inputs["Wv"], np.float32).astype(bf16)
    Wo = np.asarray(inputs["Wo"], np.float32).astype(bf16)
    bq32 = np.asarray(inputs["bq"], np.float32)
    bk32 = np.asarray(inputs["bk"], np.float32)
    bqr = np.ascontiguousarray(bq32.reshape(8, 128).T)
    bkr = np.ascontiguousarray(bk32.reshape(8, 128).T)
    bv = np.asarray(inputs["bv"], np.float32).astype(bf16).reshape(1, D)
    bo = np.asarray(inputs["bo"], np.float32).astype(bf16).reshape(1, D)

    kT = [np.ascontiguousarray(k[b].T).astype(bf16) for b in range(B)]
    vT = [np.ascontiguousarray(v[b].T).astype(bf16) for b in range(B)]

    in_maps = []
    for c in range(8):
        b, qq = c // 4, (c % 4) * SQ
        qT = np.ascontiguousarray(q[b, qq : qq + SQ, :].T).astype(bf16)
        in_maps.append(
            {
                "qT": qT,
                "kT": kT[b],
                "vT": vT[b],
                "wq": Wq,
                "wk": Wk,
                "wv": Wv,
                "wo": Wo,
                "bqr": bqr,
                "bkr": bkr,
                "bv": bv,
                "bo": bo,
            }
        )
    return in_maps


def run(inputs, trace=False, trace_kwargs=None):
    """Build (cached), run on 8 cores, return (output, BassKernelResults)."""
    from concourse.bass_utils import run_bass_kernel_spmd

    if "nc" not in _CACHE:
        _CACHE["nc"] = _build()
    nc = _CACHE["nc"]
    in_maps = _prep_inputs(inputs)
    res = run_bass_kernel_spmd(
        nc,
        in_maps,
        core_ids=list(range(8)),
        trace=trace,
        **(trace_kwargs or {}),
    )
    out = np.empty((B, S, D), np.float32)
    for c in range(8):
        b, qq = c // 4, (c % 4) * SQ
        out[b, qq : qq + SQ, :] = res.results[c]["out"]
    return out, res


def kernel(**inputs) -> np.ndarray:
    out, _ = run(inputs, trace=False)
    return out


# revision 17
# speedup vs baseline: 1.2378x; 1.0609x over previous
"""Trainium2 Bass kernel for nn_MultiHeadAttention_22883585753377.

Reference semantics (torch legacy): softmax over the HEADS axis (dim=1) of
the [B,H,S,S] score tensor, scale = sqrt(KEY_DIM)=32.

Sharding: 8 cores = (batch b, query-quarter). Each core handles b = c//4 and
512 query rows, all 16 heads (the heads-softmax couples heads, so they stay
local).

v2: single fused pass. K/V projections are produced just-in-time inside the
first query-half's attention loop so their matmuls overlap the exp (scalar)
and softmax-tree/multiply (vector) work of earlier k-chunks. Scores use one
matmul per head-PAIR (stationary = K^T pair chunk [128 feat, 128 k], moving =
zero-padded Q pair tile [128, 512]) instead of two per head. PSUM: 4 banks of
AV accumulators + 2 rotating [128,1024] tiles shared by scores, projections,
and the output projection.
"""

import numpy as np

B = 2
S = 1024 * 2
D = 1024
H = 16
DH = 64
SQ = 512  # query rows per core
QH = 256  # q processed per half
KC = 128  # k-chunk (partition dim of scores^T tiles)
NKC = S // KC  # 16
KG = 4  # k-chunks per jit projection group
NG = NKC // KG  # 4 groups of 512 k/s rows
SCALE = 1.0 / 32.0  # 1/sqrt(KEY_DIM)

_CACHE = {}


def _legalize_waits(nc):
    """This container's walrus encodes at most ONE semaphore wait per
    instruction; Tile emits up to ~10. Split the excess onto same-engine nops
    inserted immediately before the instruction."""
    import bass_rust

    ctr = [0]
    for bb in nc.main_func.blocks:
        insts = list(bb.instructions)
        out = []
        changed = False
        for ins in insts:
            si = ins.sync_info
            waits = list(si.on_wait) if si is not None and si.on_wait else []
            if len(waits) > 1:
                changed = True
                upd = list(si.on_update) if si.on_update else []
                for w in waits[:-1]:
                    ctr[0] += 1
                    nop = bass_rust.InstNoOp(
                        name=f"I-wsplit-{ctr[0]}", ins=[], outs=[]
                    )
                    nop.engine = ins.engine
                    nop.bass_nofuse = True
                    nop.sync_info = bass_rust.SyncInfo(on_wait=[w], on_update=[])
                    out.append(nop)
                ins.sync_info = bass_rust.SyncInfo(
                    on_wait=[waits[-1]], on_update=upd
                )
            out.append(ins)
        if changed:
            bb.instructions = out


def _build(legalize=True):
    import concourse.bass as bass
    import concourse.mybir as mybir
    import concourse.tile as tile

    bf16 = mybir.dt.bfloat16
    f32 = mybir.dt.float32
    AF = mybir.ActivationFunctionType

    nc = bass.Bass()

    # --- I/O ---------------------------------------------------------------
    qT_d = nc.dram_tensor("qT", [D, SQ], bf16, kind="ExternalInput")
    kT_d = nc.dram_tensor("kT", [D, S], bf16, kind="ExternalInput")
    vT_d = nc.dram_tensor("vT", [D, S], bf16, kind="ExternalInput")
    wq_d = nc.dram_tensor("wq", [D, D], bf16, kind="ExternalInput")
    wk_d = nc.dram_tensor("wk", [D, D], bf16, kind="ExternalInput")
    wv_d = nc.dram_tensor("wv", [D, D], bf16, kind="ExternalInput")
    wo_d = nc.dram_tensor("wo", [D, D], bf16, kind="ExternalInput")
    bqr_d = nc.dram_tensor("bqr", [128, 8], f32, kind="ExternalInput")
    bkr_d = nc.dram_tensor("bkr", [128, 8], f32, kind="ExternalInput")
    bv_d = nc.dram_tensor("bv", [1, D], bf16, kind="ExternalInput")
    bo_d = nc.dram_tensor("bo", [1, D], bf16, kind="ExternalInput")
    out_d = nc.dram_tensor("out", [SQ, D], f32, kind="ExternalOutput")

    with tile.TileContext(nc) as tc:
        with (
            tc.tile_pool(name="persist", bufs=1) as persist,
            tc.tile_pool(name="consts", bufs=1) as consts,
            tc.tile_pool(name="wkv", bufs=1) as wkv,
            tc.tile_pool(name="kraw", bufs=1) as krawp,
            tc.tile_pool(name="vsl", bufs=1) as vsl,
            tc.tile_pool(name="ps", bufs=2, space="PSUM") as psp,
            tc.tile_pool(name="pav", bufs=1, space="PSUM") as pav,
            tc.tile_pool(name="exp", bufs=2) as expp,
            tc.tile_pool(name="wts", bufs=5) as wtsp,
            tc.tile_pool(name="midA", bufs=1) as midA,
            tc.tile_pool(name="midB", bufs=2) as midB,
            tc.tile_pool(name="wrow", bufs=1) as wrow,
            tc.tile_pool(name="osb", bufs=1) as osbp,
        ):
            # ---- persistent SBUF ------------------------------------------
            # K^T [D,S]: pair chunk f holds heads 2f (rows 0-63), 2f+1
            # (rows 64-127); filled group by group.
            KT = [persist.tile([128, S], bf16, tag=f"KT{f}", name=f"KT{f}") for f in range(8)]
            # V natural [S,D] as 16 x [128, D]
            V = [persist.tile([128, D], bf16, tag=f"V{s}", name=f"V{s}") for s in range(16)]
            # Q^T pair tiles [128, 2*SQ]: cols [qh0: even 256 | odd 256]
            # [qh1: even | odd]; even head rows 0-63 (64-127 zero), odd head
            # rows 64-127 (0-63 zero).
            QTP = [persist.tile([128, 2 * SQ], bf16, tag=f"QTP{f}", name=f"QTP{f}") for f in range(8)]
            # O^T [D,SQ] as 8 x [128, SQ] (pair j = heads 2j,2j+1)
            OT = [persist.tile([128, SQ], bf16, tag=f"OT{p}", name=f"OT{p}") for p in range(8)]

            ones = consts.tile([1, 128], bf16)
            nc.vector.memset(ones[:], 1.0)
            bqr_s = consts.tile([128, 8], f32, tag="bqr")
            bkr_s = consts.tile([128, 8], f32, tag="bkr")
            bv_s = consts.tile([1, D], bf16, tag="bv")
            bo_s = consts.tile([1, D], bf16, tag="bo")
            wkr = [wkv.tile([128, D], bf16, tag=f"wkr{d}", name=f"wkr{d}") for d in range(8)]
            wvr = [wkv.tile([128, D], bf16, tag=f"wvr{d}", name=f"wvr{d}") for d in range(8)]

            def load_kraw(g):
                """kraw group g: 8 tiles [128, 512] (cols g*512..)."""
                ts = []
                for d in range(8):
                    t = krawp.tile([128, 512], bf16, tag=f"kraw{d}", name=f"kraw{d}g{g}")
                    nc.sync.dma_start(t[:], kT_d[d * 128 : (d + 1) * 128, g * 512 : (g + 1) * 512])
                    ts.append(t)
                return ts

            def load_vsl(sc):
                """vT slice for s-chunk sc: 8 tiles [128, 128]."""
                ts = []
                for d in range(8):
                    t = vsl.tile([128, 128], bf16, tag=f"vsl{d}", name=f"vsl{d}s{sc}")
                    nc.sync.dma_start(t[:], vT_d[d * 128 : (d + 1) * 128, sc * 128 : (sc + 1) * 128])
                    ts.append(t)
                return ts

            def proj_k(g, fo_list, kr):
                """Project K chunks for group g, pair-features fo in fo_list."""
                bt = psp.tile([128, 512 * len(fo_list)], f32, tag="ps")
                for i, fo in enumerate(fo_list):
                    for d in range(8):
                        nc.tensor.matmul(
                            bt[:, i * 512 : (i + 1) * 512],
                            wkr[d][:, fo * 128 : (fo + 1) * 128],
                            kr[d][:],
                            start=(d == 0),
                            stop=(d == 7),
                        )
                for i, fo in enumerate(fo_list):
                    nc.scalar.activation(
                        KT[fo][:, g * 512 : (g + 1) * 512],
                        bt[:, i * 512 : (i + 1) * 512],
                        AF.Identity,
                        bias=bkr_s[:, fo : fo + 1],
                    )

            def proj_v(sc, vs):
                """Project V s-chunk sc (V[sc] = [128, D])."""
                pv = psp.tile([128, D], f32, tag="ps")
                for f2 in range(2):
                    fs = slice(f2 * 512, (f2 + 1) * 512)
                    for d in range(8):
                        nc.tensor.matmul(
                            pv[:, fs],
                            vs[d][:],
                            wvr[d][:, fs],
                            start=(d == 0),
                            stop=False,
                        )
                    nc.tensor.matmul(
                        pv[:, fs], ones[0:1, :], bv_s[0:1, fs], start=False, stop=True
                    )
                nc.scalar.copy(V[sc][:], pv[:])

            # ---- prologue: Q proj + K/V group 0 ---------------------------
            with tc.tile_pool(name="qraw", bufs=1) as qrawp:
                qraw = [qrawp.tile([128, SQ], bf16, tag=f"qraw{d}", name=f"qraw{d}") for d in range(8)]
                wqr = [wrow.tile([128, D], bf16, tag=f"w{d}", name=f"wqr{d}") for d in range(8)]
                for d in range(8):
                    nc.sync.dma_start(qraw[d][:], qT_d[d * 128 : (d + 1) * 128, :])
                    nc.sync.dma_start(wqr[d][:], wq_d[d * 128 : (d + 1) * 128, :])
                nc.sync.dma_start(bqr_s[:], bqr_d[:])
                for f in range(8):
                    nc.vector.memset(QTP[f][:], 0.0)
                kr0 = load_kraw(0)
                nc.sync.dma_start(bkr_s[:], bkr_d[:])
                for d in range(8):
                    nc.sync.dma_start(wkr[d][:], wk_d[d * 128 : (d + 1) * 128, :])
                vs_pend = load_vsl(0)
                for d in range(8):
                    nc.sync.dma_start(wvr[d][:], wv_d[d * 128 : (d + 1) * 128, :])
                nc.sync.dma_start(bv_s[:], bv_d[:])
                nc.sync.dma_start(bo_s[:], bo_d[:])

                for fp in range(4):  # feature pair-chunks: f = 2fp, 2fp+1
                    ps = psp.tile([128, 1024], f32, tag="ps")
                    for i in range(2):
                        f = 2 * fp + i
                        for d in range(8):
                            nc.tensor.matmul(
                                ps[:, i * 512 : (i + 1) * 512],
                                wqr[d][:, f * 128 : (f + 1) * 128],
                                qraw[d][:],
                                start=(d == 0),
                                stop=(d == 7),
                            )
                    for i in range(2):
                        f = 2 * fp + i
                        for qh in range(2):
                            qs = slice(i * 512 + qh * 256, i * 512 + (qh + 1) * 256)
                            nc.scalar.activation(
                                QTP[f][0:64, qh * 512 : qh * 512 + 256],
                                ps[0:64, qs],
                                AF.Identity,
                                bias=bqr_s[0:64, f : f + 1],
                            )
                            nc.scalar.activation(
                                QTP[f][64:128, qh * 512 + 256 : (qh + 1) * 512],
                                ps[64:128, qs],
                                AF.Identity,
                                bias=bqr_s[64:128, f : f + 1],
                            )

                # K group 0: 8 pair-features in 4 borrowed tiles
                for fp in range(4):
                    proj_k(0, [2 * fp, 2 * fp + 1], kr0)
                # V group 0 (s-chunks 0-3)
                for sc in range(4):
                    vs = vs_pend
                    if sc < 3:
                        vs_pend = load_vsl(sc + 1)
                    proj_v(sc, vs)

            # ---- attention ------------------------------------------------
            kr_pend = load_kraw(1)
            wo_t = [wrow.tile([128, D], bf16, tag=f"w{j}", name=f"wo{j}") for j in range(8)]
            for j in range(8):
                nc.sync.dma_start(wo_t[j][:], wo_d[j * 128 : (j + 1) * 128, :])

            def oproj(q4, osb):
                qsl = slice(q4 * 128, (q4 + 1) * 128)
                po = psp.tile([128, D], f32, tag="ps")
                for f2 in range(2):
                    fs = slice(f2 * 512, (f2 + 1) * 512)
                    for j in range(8):
                        nc.tensor.matmul(
                            po[:, fs],
                            OT[j][:, qsl],
                            wo_t[j][:, fs],
                            start=(j == 0),
                            stop=False,
                        )
                    nc.tensor.matmul(
                        po[:, fs], ones[0:1, :], bo_s[0:1, fs], start=False, stop=True
                    )
                ob = osb.tile([128, D], f32, tag="ob")
                nc.vector.tensor_copy(ob[:], po[:])
                nc.gpsimd.dma_start(out_d[qsl, :], ob[:])

            for qh in range(2):
                qsl = slice(qh * QH, (qh + 1) * QH)
                oacc = [
                    pav.tile([128, 2 * QH], f32, tag=f"oacc{i}", name=f"oacc{i}q{qh}", bufs=1)
                    for i in range(4)
                ]
                for kc in range(NKC):
                    # scores + exp: 2 pairs per rotating psum tile
                    kcs = slice(kc * 128, (kc + 1) * 128)
                    e = expp.tile([128, H * QH], bf16, tag="e")
                    for g2 in range(4):
                        sc2 = psp.tile([128, 1024], f32, tag="ps")
                        for i in range(2):
                            f = 2 * g2 + i
                            nc.tensor.matmul(
                                sc2[:, i * 512 : (i + 1) * 512],
                                KT[f][:, kcs],
                                QTP[f][:, qh * 512 : (qh + 1) * 512],
                                start=True,
                                stop=True,
                            )
                        nc.scalar.activation(
                            e[:, g2 * 1024 : (g2 + 1) * 1024],
                            sc2[:],
                            AF.Exp,
                            scale=SCALE,
                        )
                    # JIT K/V projection for group kc//4 + 1 (qh0 only)
                    if qh == 0 and kc < 12:
                        g = kc // 4 + 1
                        s = kc % 4
                        if s == 0:
                            kr_cur = kr_pend
                        proj_k(g, [2 * s, 2 * s + 1], kr_cur)
                        vs = vs_pend
                        if not (g == 3 and s == 3):
                            vs_pend = load_vsl(4 * g + s + 1)
                        proj_v(4 * g + s, vs)
                        if s == 3 and g < 3:
                            kr_pend = load_kraw(g + 1)

                    # denominator over heads (tree), then 1/den
                    t1 = midA.tile([128, 8 * QH], bf16, tag="t1")
                    nc.vector.tensor_add(t1[:], e[:, : 8 * QH], e[:, 8 * QH :])
                    t2 = midA.tile([128, 4 * QH], bf16, tag="t2")
                    nc.vector.tensor_add(t2[:], t1[:, : 4 * QH], t1[:, 4 * QH :])
                    t3 = midA.tile([128, 2 * QH], bf16, tag="t3")
                    nc.vector.tensor_add(t3[:], t2[:, : 2 * QH], t2[:, 2 * QH :])
                    den = midB.tile([128, QH], bf16, tag="den")
                    nc.vector.tensor_add(den[:], t3[:, :QH], t3[:, QH:])
                    lden = midB.tile([128, QH], f32, tag="lden")
                    nc.scalar.activation(lden[:], den[:], AF.Ln)
                    rb = midB.tile([128, 4 * QH], bf16, tag="rb")
                    nc.scalar.activation(rb[:, :QH], lden[:], AF.Exp, scale=-1.0)
                    nc.vector.tensor_copy(rb[:, QH : 2 * QH], rb[:, :QH])
                    nc.vector.tensor_copy(rb[:, 2 * QH :], rb[:, : 2 * QH])
                    # weights = e * (1/den), per 2-pair chunk
                    wch = []
                    for g2 in range(4):
                        gs = slice(g2 * 4 * QH, (g2 + 1) * 4 * QH)
                        w = wtsp.tile([128, 4 * QH], bf16, tag="w")
                        nc.vector.tensor_mul(w[:], e[:, gs], rb[:])
                        wch.append(w)
                    # AV: O^T[pair] += V_h^T-slice x w_h
                    for j in range(8):
                        cs = slice((j // 4) * QH, (j // 4 + 1) * QH)
                        for hh in range(2):
                            h = 2 * j + hh
                            nc.tensor.matmul(
                                oacc[j % 4][hh * 64 : (hh + 1) * 64, cs],
                                V[kc][:, h * 64 : (h + 1) * 64],
                                wch[j // 2][:, (j % 2) * 512 + hh * 256 : (j % 2) * 512 + (hh + 1) * 256],
                                start=(kc == 0 and j < 4),
                                stop=(kc == NKC - 1),
                                skip_group_check=True,
                            )
                for j in range(8):
                    cs = slice((j // 4) * QH, (j // 4 + 1) * QH)
                    nc.scalar.copy(OT[j][:, qsl], oacc[j % 4][:, cs])
                oproj(2 * qh, osbp)
                oproj(2 * qh + 1, osbp)


    if legalize:
        _legalize_waits(nc)
    return nc


def _prep_inputs(inputs):
    import ml_dtypes

    bf16 = ml_dtypes.bfloat16
    q = np.asarray(inputs["queries"], np.float32)
    k = np.asarray(inputs["keys"], np.float32)
    v = np.asarray(inputs["values"], np.float32)
    Wq = np.asarray(inputs["Wq"], np.float32).astype(bf16)
    Wk = np.asarray(inputs["Wk"], np.float32).astype(bf16)
    Wv = np.asarray(inputs["Wv"], np.float32).astype(bf16)
    Wo = np.asarray(inputs["Wo"], np.float32).astype(bf16)
    bq32 = np.asarray(inputs["bq"], np.float32)
    bk32 = np.asarray(inputs["bk"], np.float32)
    bqr = np.ascontiguousarray(bq32.reshape(8, 128).T)
    bkr = np.ascontiguousarray(bk32.reshape(8, 128).T)
    bv = np.asarray(inputs["bv"], np.float32).astype(bf16).reshape(1, D)
    bo = np.asarray(inputs["bo"], np.float32).astype(bf16).reshape(1, D)

    kT = [np.ascontiguousarray(k[b].T).astype(bf16) for b in range(B)]
    vT = [np.ascontiguousarray(v[b].T).astype(bf16) for b in range(B)]

    in_maps = []
    for c in range(8):
        b, qq = c // 4, (c % 4) * SQ
        qT = np.ascontiguousarray(q[b, qq : qq + SQ, :].T).astype(bf16)
        in_maps.append(
            {
                "qT": qT,
                "kT": kT[b],
                "vT": vT[b],
                "wq": Wq,
                "wk": Wk,
                "wv": Wv,
                "wo": Wo,
                "bqr": bqr,
                "bkr": bkr,
                "bv": bv,
                "bo": bo,
            }
        )
    return in_maps


def run(inputs, trace=False, trace_kwargs=None):
    """Build (cached), run on 8 cores, return (output, BassKernelResults)."""
    from concourse.bass_utils import run_bass_kernel_spmd

    if "nc" not in _CACHE:
        _CACHE["nc"] = _build()
    nc = _CACHE["nc"]
    in_maps = _prep_inputs(inputs)
    res = run_bass_kernel_spmd(
        nc,
        in_maps,
        core_ids=list(range(8)),
        trace=trace,
        **(trace_kwargs or {}),
    )
    out = np.empty((B, S, D), np.float32)
    for c in range(8):
        b, qq = c // 4, (c % 4) * SQ
        out[b, qq : qq + SQ, :] = res.results[c]["out"]
    return out, res


def kernel(**inputs) -> np.ndarray:
    out, _ = run(inputs, trace=False)
    return out


# revision 19
# speedup vs baseline: 1.2418x; 1.0032x over previous
"""Trainium2 Bass kernel for nn_MultiHeadAttention_22883585753377.

Reference semantics (torch legacy): softmax over the HEADS axis (dim=1) of
the [B,H,S,S] score tensor, scale = sqrt(KEY_DIM)=32.

Sharding: 8 cores = (batch b, query-quarter). Each core handles b = c//4 and
512 query rows, all 16 heads (the heads-softmax couples heads, so they stay
local).

v2: single fused pass. K/V projections are produced just-in-time inside the
first query-half's attention loop so their matmuls overlap the exp (scalar)
and softmax-tree/multiply (vector) work of earlier k-chunks. Scores use one
matmul per head-PAIR (stationary = K^T pair chunk [128 feat, 128 k], moving =
zero-padded Q pair tile [128, 512]) instead of two per head. PSUM: 4 banks of
AV accumulators + 2 rotating [128,1024] tiles shared by scores, projections,
and the output projection.
"""

import numpy as np

B = 2
S = 1024 * 2
D = 1024
H = 16
DH = 64
SQ = 512  # query rows per core
QH = 256  # q processed per half
KC = 128  # k-chunk (partition dim of scores^T tiles)
NKC = S // KC  # 16
KG = 4  # k-chunks per jit projection group
NG = NKC // KG  # 4 groups of 512 k/s rows
SCALE = 1.0 / 32.0  # 1/sqrt(KEY_DIM)

_CACHE = {}


def _legalize_waits(nc):
    """This container's walrus encodes at most ONE semaphore wait per
    instruction; Tile emits up to ~10. Split the excess onto same-engine nops
    inserted immediately before the instruction."""
    import bass_rust

    ctr = [0]
    for bb in nc.main_func.blocks:
        insts = list(bb.instructions)
        out = []
        changed = False
        for ins in insts:
            si = ins.sync_info
            waits = list(si.on_wait) if si is not None and si.on_wait else []
            if len(waits) > 1:
                changed = True
                upd = list(si.on_update) if si.on_update else []
                for w in waits[:-1]:
                    ctr[0] += 1
                    nop = bass_rust.InstNoOp(
                        name=f"I-wsplit-{ctr[0]}", ins=[], outs=[]
                    )
                    nop.engine = ins.engine
                    nop.bass_nofuse = True
                    nop.sync_info = bass_rust.SyncInfo(on_wait=[w], on_update=[])
                    out.append(nop)
                ins.sync_info = bass_rust.SyncInfo(
                    on_wait=[waits[-1]], on_update=upd
                )
            out.append(ins)
        if changed:
            bb.instructions = out


def _build(legalize=True):
    import concourse.bass as bass
    import concourse.mybir as mybir
    import concourse.tile as tile

    bf16 = mybir.dt.bfloat16
    f32 = mybir.dt.float32
    AF = mybir.ActivationFunctionType

    nc = bass.Bass()

    # --- I/O ---------------------------------------------------------------
    qT_d = nc.dram_tensor("qT", [D, SQ], bf16, kind="ExternalInput")
    kT_d = nc.dram_tensor("kT", [D, S], bf16, kind="ExternalInput")
    vT_d = nc.dram_tensor("vT", [D, S], bf16, kind="ExternalInput")
    wq_d = nc.dram_tensor("wq", [D, D], bf16, kind="ExternalInput")
    wk_d = nc.dram_tensor("wk", [D, D], bf16, kind="ExternalInput")
    wv_d = nc.dram_tensor("wv", [D, D], bf16, kind="ExternalInput")
    wo_d = nc.dram_tensor("wo", [D, D], bf16, kind="ExternalInput")
    bqr_d = nc.dram_tensor("bqr", [128, 8], f32, kind="ExternalInput")
    bkr_d = nc.dram_tensor("bkr", [128, 8], f32, kind="ExternalInput")
    bv_d = nc.dram_tensor("bv", [1, D], bf16, kind="ExternalInput")
    bo_d = nc.dram_tensor("bo", [1, D], bf16, kind="ExternalInput")
    out_d = nc.dram_tensor("out", [SQ, D], f32, kind="ExternalOutput")

    with tile.TileContext(nc) as tc:
        with (
            tc.tile_pool(name="persist", bufs=1) as persist,
            tc.tile_pool(name="consts", bufs=1) as consts,
            tc.tile_pool(name="wkv", bufs=1) as wkv,
            tc.tile_pool(name="kraw", bufs=1) as krawp,
            tc.tile_pool(name="vsl", bufs=1) as vsl,
            tc.tile_pool(name="ps", bufs=2, space="PSUM") as psp,
            tc.tile_pool(name="pav", bufs=1, space="PSUM") as pav,
            tc.tile_pool(name="exp", bufs=2) as expp,
            tc.tile_pool(name="wts", bufs=5) as wtsp,
            tc.tile_pool(name="midA", bufs=1) as midA,
            tc.tile_pool(name="midB", bufs=2) as midB,
            tc.tile_pool(name="wrow", bufs=1) as wrow,
            tc.tile_pool(name="osb", bufs=1) as osbp,
        ):
            # ---- persistent SBUF ------------------------------------------
            # K^T [D,S]: pair chunk f holds heads 2f (rows 0-63), 2f+1
            # (rows 64-127); filled group by group.
            KT = [persist.tile([128, S], bf16, tag=f"KT{f}", name=f"KT{f}") for f in range(8)]
            # V natural [S,D] as 16 x [128, D]
            V = [persist.tile([128, D], bf16, tag=f"V{s}", name=f"V{s}") for s in range(16)]
            # Q^T pair tiles [128, 2*SQ]: cols [qh0: even 256 | odd 256]
            # [qh1: even | odd]; even head rows 0-63 (64-127 zero), odd head
            # rows 64-127 (0-63 zero).
            QTP = [persist.tile([128, 2 * SQ], bf16, tag=f"QTP{f}", name=f"QTP{f}") for f in range(8)]
            # O^T [D,SQ] as 8 x [128, SQ] (pair j = heads 2j,2j+1)
            OT = [persist.tile([128, SQ], bf16, tag=f"OT{p}", name=f"OT{p}") for p in range(8)]

            ones = consts.tile([1, 128], bf16)
            nc.vector.memset(ones[:], 1.0)
            bqr_s = consts.tile([128, 8], f32, tag="bqr")
            bkr_s = consts.tile([128, 8], f32, tag="bkr")
            bv_s = consts.tile([1, D], bf16, tag="bv")
            bo_s = consts.tile([1, D], bf16, tag="bo")
            wkr = [wkv.tile([128, D], bf16, tag=f"wkr{d}", name=f"wkr{d}") for d in range(8)]
            wvr = [wkv.tile([128, D], bf16, tag=f"wvr{d}", name=f"wvr{d}") for d in range(8)]

            def load_kraw(g):
                """kraw group g: 8 tiles [128, 512] (cols g*512..)."""
                ts = []
                for d in range(8):
                    t = krawp.tile([128, 512], bf16, tag=f"kraw{d}", name=f"kraw{d}g{g}")
                    nc.sync.dma_start(t[:], kT_d[d * 128 : (d + 1) * 128, g * 512 : (g + 1) * 512])
                    ts.append(t)
                return ts

            def load_vsl(sc):
                """vT slice for s-chunk sc: 8 tiles [128, 128]."""
                ts = []
                for d in range(8):
                    t = vsl.tile([128, 128], bf16, tag=f"vsl{d}", name=f"vsl{d}s{sc}")
                    nc.sync.dma_start(t[:], vT_d[d * 128 : (d + 1) * 128, sc * 128 : (sc + 1) * 128])
                    ts.append(t)
                return ts

            def proj_k(g, fo_list, kr):
                """Project K chunks for group g, pair-features fo in fo_list."""
                bt = psp.tile([128, 512 * len(fo_list)], f32, tag="ps")
                for i, fo in enumerate(fo_list):
                    for d in range(8):
                        nc.tensor.matmul(
                            bt[:, i * 512 : (i + 1) * 512],
                            wkr[d][:, fo * 128 : (fo + 1) * 128],
                            kr[d][:],
                            start=(d == 0),
                            stop=(d == 7),
                        )
                for i, fo in enumerate(fo_list):
                    nc.scalar.activation(
                        KT[fo][:, g * 512 : (g + 1) * 512],
                        bt[:, i * 512 : (i + 1) * 512],
                        AF.Identity,
                        bias=bkr_s[:, fo : fo + 1],
                    )

            def proj_v(sc, vs):
                """Project V s-chunk sc (V[sc] = [128, D])."""
                pv = psp.tile([128, D], f32, tag="ps")
                for f2 in range(2):
                    fs = slice(f2 * 512, (f2 + 1) * 512)
                    for d in range(8):
                        nc.tensor.matmul(
                            pv[:, fs],
                            vs[d][:],
                            wvr[d][:, fs],
                            start=(d == 0),
                            stop=False,
                        )
                    nc.tensor.matmul(
                        pv[:, fs], ones[0:1, :], bv_s[0:1, fs], start=False, stop=True
                    )
                nc.scalar.copy(V[sc][:], pv[:])

            # ---- prologue: Q proj + K/V group 0 ---------------------------
            with tc.tile_pool(name="qraw", bufs=1) as qrawp:
                qraw = [qrawp.tile([128, SQ], bf16, tag=f"qraw{d}", name=f"qraw{d}") for d in range(8)]
                wqr = [wrow.tile([128, D], bf16, tag=f"w{d}", name=f"wqr{d}") for d in range(8)]
                for d in range(8):
                    nc.sync.dma_start(qraw[d][:], qT_d[d * 128 : (d + 1) * 128, :])
                    nc.sync.dma_start(wqr[d][:], wq_d[d * 128 : (d + 1) * 128, :])
                nc.sync.dma_start(bqr_s[:], bqr_d[:])
                for f in range(8):
                    nc.vector.memset(QTP[f][:], 0.0)
                kr0 = load_kraw(0)
                nc.sync.dma_start(bkr_s[:], bkr_d[:])
                for d in range(8):
                    nc.sync.dma_start(wkr[d][:], wk_d[d * 128 : (d + 1) * 128, :])
                vs_pend = load_vsl(0)
                for d in range(8):
                    nc.sync.dma_start(wvr[d][:], wv_d[d * 128 : (d + 1) * 128, :])
                nc.sync.dma_start(bv_s[:], bv_d[:])
                nc.sync.dma_start(bo_s[:], bo_d[:])

                for fp in range(4):  # feature pair-chunks: f = 2fp, 2fp+1
                    ps = psp.tile([128, 1024], f32, tag="ps")
                    for i in range(2):
                        f = 2 * fp + i
                        for d in range(8):
                            nc.tensor.matmul(
                                ps[:, i * 512 : (i + 1) * 512],
                                wqr[d][:, f * 128 : (f + 1) * 128],
                                qraw[d][:],
                                start=(d == 0),
                                stop=(d == 7),
                            )
                    for i in range(2):
                        f = 2 * fp + i
                        for qh in range(2):
                            qs = slice(i * 512 + qh * 256, i * 512 + (qh + 1) * 256)
                            nc.scalar.activation(
                                QTP[f][0:64, qh * 512 : qh * 512 + 256],
                                ps[0:64, qs],
                                AF.Identity,
                                bias=bqr_s[0:64, f : f + 1],
                            )
                            nc.scalar.activation(
                                QTP[f][64:128, qh * 512 + 256 : (qh + 1) * 512],
                                ps[64:128, qs],
                                AF.Identity,
                                bias=bqr_s[64:128, f : f + 1],
                            )

                # K group 0: 8 pair-features in 4 borrowed tiles
                for fp in range(4):
                    proj_k(0, [2 * fp, 2 * fp + 1], kr0)
                # V group 0 (s-chunks 0-3)
                for sc in range(4):
                    vs = vs_pend
                    if sc < 3:
                        vs_pend = load_vsl(sc + 1)
                    proj_v(sc, vs)

            # ---- attention ------------------------------------------------
            kr_pend = load_kraw(1)
            wo_t = [wrow.tile([128, D], bf16, tag=f"w{j}", name=f"wo{j}") for j in range(8)]
            for j in range(8):
                nc.sync.dma_start(wo_t[j][:], wo_d[j * 128 : (j + 1) * 128, :])

            def oproj(q4, osb):
                qsl = slice(q4 * 128, (q4 + 1) * 128)
                po = psp.tile([128, D], f32, tag="ps")
                for f2 in range(2):
                    fs = slice(f2 * 512, (f2 + 1) * 512)
                    for j in range(8):
                        nc.tensor.matmul(
                            po[:, fs],
                            OT[j][:, qsl],
                            wo_t[j][:, fs],
                            start=(j == 0),
                            stop=False,
                        )
                    nc.tensor.matmul(
                        po[:, fs], ones[0:1, :], bo_s[0:1, fs], start=False, stop=True
                    )
                ob = osb.tile([128, D], f32, tag="ob")
                nc.vector.tensor_copy(ob[:], po[:])
                nc.gpsimd.dma_start(out_d[qsl, :], ob[:])

            for qh in range(2):
                qsl = slice(qh * QH, (qh + 1) * QH)
                oacc = [
                    pav.tile([128, 2 * QH], f32, tag=f"oacc{i}", name=f"oacc{i}q{qh}", bufs=1)
                    for i in range(4)
                ]
                for kc in range(NKC):
                    # scores + exp: 2 pairs per rotating psum tile
                    kcs = slice(kc * 128, (kc + 1) * 128)
                    e = expp.tile([128, H * QH], bf16, tag="e")
                    for g2 in range(4):
                        sc2 = psp.tile([128, 1024], f32, tag="ps")
                        for i in range(2):
                            f = 2 * g2 + i
                            nc.tensor.matmul(
                                sc2[:, i * 512 : (i + 1) * 512],
                                KT[f][:, kcs],
                                QTP[f][:, qh * 512 : (qh + 1) * 512],
                                start=True,
                                stop=True,
                            )
                        nc.scalar.activation(
                            e[:, g2 * 1024 : (g2 + 1) * 1024],
                            sc2[:],
                            AF.Exp,
                            scale=SCALE,
                        )
                    # JIT K/V projection for group kc//4 + 1 (qh0 only)
                    if qh == 0 and kc < 12:
                        g = kc // 4 + 1
                        s = kc % 4
                        if s == 0:
                            kr_cur = kr_pend
                        proj_k(g, [2 * s, 2 * s + 1], kr_cur)
                        vs = vs_pend
                        if not (g == 3 and s == 3):
                            vs_pend = load_vsl(4 * g + s + 1)
                        proj_v(4 * g + s, vs)
                        if s == 3 and g < 3:
                            kr_pend = load_kraw(g + 1)

                    # denominator over heads (tree), then 1/den
                    t1 = midA.tile([128, 8 * QH], bf16, tag="t1")
                    nc.vector.tensor_add(t1[:], e[:, : 8 * QH], e[:, 8 * QH :])
                    t2 = midA.tile([128, 4 * QH], bf16, tag="t2")
                    nc.vector.tensor_add(t2[:], t1[:, : 4 * QH], t1[:, 4 * QH :])
                    t3 = midA.tile([128, 2 * QH], bf16, tag="t3")
                    nc.vector.tensor_add(t3[:], t2[:, : 2 * QH], t2[:, 2 * QH :])
                    den = midB.tile([128, QH], bf16, tag="den")
                    nc.vector.tensor_add(den[:], t3[:, :QH], t3[:, QH:])
                    lden = midB.tile([128, QH], f32, tag="lden")
                    nc.scalar.activation(lden[:], den[:], AF.Ln)
                    rb = midB.tile([128, 4 * QH], bf16, tag="rb")
                    nc.scalar.activation(rb[:, :QH], lden[:], AF.Exp, scale=-1.0)
                    nc.vector.tensor_copy(rb[:, QH : 2 * QH], rb[:, :QH])
                    nc.vector.tensor_copy(rb[:, 2 * QH :], rb[:, : 2 * QH])
                    # weights = e * (1/den), per 2-pair chunk
                    wch = []
                    for g2 in range(4):
                        gs = slice(g2 * 4 * QH, (g2 + 1) * 4 * QH)
                        w = wtsp.tile([128, 4 * QH], bf16, tag="w")
                        nc.vector.tensor_mul(w[:], e[:, gs], rb[:])
                        wch.append(w)
                    # AV: O^T[pair] += V_h^T-slice x w_h
                    for j in range(8):
                        cs = slice((j // 4) * QH, (j // 4 + 1) * QH)
                        for hh in range(2):
                            h = 2 * j + hh
                            nc.tensor.matmul(
                                oacc[j % 4][hh * 64 : (hh + 1) * 64, cs],
                                V[kc][:, h * 64 : (h + 1) * 64],
                                wch[j // 2][:, (j % 2) * 512 + hh * 256 : (j % 2) * 512 + (hh + 1) * 256],
                                start=(kc == 0 and j < 4),
                                stop=(kc == NKC - 1),
                                skip_group_check=True,
                            )
                for j in range(8):
                    cs = slice((j // 4) * QH, (j // 4 + 1) * QH)
                    nc.scalar.copy(OT[j][:, qsl], oacc[j % 4][:, cs])
                oproj(2 * qh, osbp)
                oproj(2 * qh + 1, osbp)


    if legalize:
        _legalize_waits(nc)
    return nc


def _prep_inputs(inputs):
    import ml_dtypes

    bf16 = ml_dtypes.bfloat16
    q = np.asarray(inputs["queries"], np.float32)
    k = np.asarray(inputs["keys"], np.float32)
    v = np.asarray(inputs["values"], np.float32)
    Wq = np.asarray(inputs["Wq"], np.float32).astype(bf16)
    Wk = np.asarray(inputs["Wk"], np.float32).astype(bf16)
    Wv = np.asarray(inputs["Wv"], np.float32).astype(bf16)
    Wo = np.asarray(inputs["Wo"], np.float32).astype(bf16)
    bq32 = np.asarray(inputs["bq"], np.float32)
    bk32 = np.asarray(inputs["bk"], np.float32)
    bqr = np.ascontiguousarray(bq32.reshape(8, 128).T)
    bkr = np.ascontiguousarray(bk32.reshape(8, 128).T)
    bv = np.asarray(inputs["bv"], np.float32).astype(bf16).reshape(1, D)
    bo = np.asarray(inputs["bo"], np.float32).astype(bf16).reshape(1, D)

    kT = [np.ascontiguousarray(k[b].T).astype(bf16) for b in range(B)]
    vT = [np.ascontiguousarray(v[b].T).astype(bf16) for b in range(B)]

    in_maps = []
    for c in range(8):
        b, qq = c // 4, (c % 4) * SQ
        qT = np.ascontiguousarray(q[b, qq : qq + SQ, :].T).astype(bf16)
        in_maps.append(
            {
                "qT": qT,
                "kT": kT[b],
                "vT": vT[b],
                "wq": Wq,
                "wk": Wk,
                "wv": Wv,
                "wo": Wo,
                "bqr": bqr,
                "bkr": bkr,
                "bv": bv,
                "bo": bo,
            }
        )
    return in_maps


def run(inputs, trace=False, trace_kwargs=None):
    """Build (cached), run on 8 cores, return (output, BassKernelResults)."""
    from concourse.bass_utils import run_bass_kernel_spmd

    if "nc" not in _CACHE:
        _CACHE["nc"] = _build()
    nc = _CACHE["nc"]
    in_maps = _prep_inputs(inputs)
    res = run_bass_kernel_spmd(
        nc,
        in_maps,
        core_ids=list(range(8)),
        trace=trace,
        **(trace_kwargs or {}),
    )
    out = np.empty((B, S, D), np.float32)
    for c in range(8):
        b, qq = c // 4, (c % 4) * SQ
        out[b, qq : qq + SQ, :] = res.results[c]["out"]
    return out, res


def kernel(**inputs) -> np.ndarray:
    out, _ = run(inputs, trace=False)
    return out
